# revision 1
# baseline (speedup 1.0000x reference)
"""Trainium2 Bass kernel for GQA multi-head attention block (nn_MHA_68831145886222).

Computation (reference):
  qkv = x @ w_qkv.T ; split q[32 heads],k[8],v[8] (HD=128)
  q,k = rmsnorm(head_dim) -> rope(interleaved, theta=1e6)
  out = causal GQA attention (4 q heads per kv head)
  y   = (attn out) @ w_out.T

Sharding: tensor-parallel by kv-head group. Core g of 8 owns q heads
4g..4g+3 and kv head g (columns of the qkv projection), plus the matching
512 input rows of w_out. Each core computes a partial y [2048,4096]; the
host sums the 8 partials.

Device-side layout choices per core:
  stage 1 (qkv proj):  stationary = x^T tiles [128 d, 128 s] (bf16),
                       moving = w_qkv^T slices -> qkv in natural [s, e] psum
  postproc: rmsnorm stats via ACT Square+accum; rope via pair-swap copy +
            two muls + add (tables host-precomputed); per-head rstd apply
            (score scale 1/sqrt(HD) and ln weights folded in); PE-transpose
            q,k to [hd, s]; v stays natural [s, hd].
  attention: scoresT [k, q] = kT-tile.T @ qT (exact causal via restricted
             moving dim); exp on ACT; diagonal 128x128 blocks masked by a
             0/1 mask mul; PV and the softmax denominator (ones-matmul)
             accumulate over k-tiles in PSUM; normalize after PV.
  stage 3 (out proj): stationary = attnT [128 hd, 128 s], moving = w_out^T
             slices; accumulate over the core's 4 heads; fp32 partial out.
"""

import os
import sys
import types

import numpy as np

H = 32
G = 8
HD = 128
S = 2048
D = 4096
HG = H // G  # q heads per kv head = 4
EPS = 1e-5
THETA = 1e6
N_CORES = 8
ST = S // 128  # 16 s-tiles
DT = D // 128  # 32 d-tiles
QC = 4  # q chunks of 512
EC = 8  # e chunks of 512 in final matmul


def _ensure_ntff_hook():
    """Register the axon NTFF profile hook if the image's antenv lacks it,
    so run_bass_kernel_spmd(trace=True) can return exec_time_ns."""
    try:
        from antenv.axon_hooks import get_axon_ntff_profile_hook  # noqa: F401
        return
    except ImportError:
        pass
    try:
        import antenv
        mod = types.ModuleType("antenv.axon_hooks")
        _h = [None]
        mod.set_axon_ntff_profile_hook = lambda h: _h.__setitem__(0, h)
        mod.get_axon_ntff_profile_hook = lambda: _h[0]
        sys.modules["antenv.axon_hooks"] = mod
        antenv.axon_hooks = mod
        from trn_agent_boot.trn_boot import _ntff_profile_via_ctypes
        so = "/opt/axon/libaxon_pjrt.so"
        if os.path.exists(so):
            mod.set_axon_ntff_profile_hook(_ntff_profile_via_ctypes(so))
    except Exception:
        pass


def _build_nc():
    import concourse.bass as bass  # noqa: F401
    import concourse.tile as tile
    from concourse import bacc, mybir

    bf16 = mybir.dt.bfloat16
    f16 = mybir.dt.float16
    f32 = mybir.dt.float32
    i32 = mybir.dt.int32
    AF = mybir.ActivationFunctionType

    nc = bacc.Bacc("TRN2", target_bir_lowering=False, debug=False,
                   num_devices=N_CORES)

    # ---- DRAM I/O ----
    xt_d = nc.dram_tensor("xt", [ST, 128, DT, 128], bf16, kind="ExternalInput").ap()
    wqkv_d = nc.dram_tensor("wqkvT", [D, 768], bf16, kind="ExternalInput").ap()
    wo_d = nc.dram_tensor("woT", [512, D], bf16, kind="ExternalInput").ap()
    ccd_d = nc.dram_tensor("ccd", [S, 128], f16, kind="ExternalInput").ap()
    ssd_d = nc.dram_tensor("ssd", [S, 128], f16, kind="ExternalInput").ap()
    mask_d = nc.dram_tensor("dmask", [128, 128], bf16, kind="ExternalInput").ap()
    ident_d = nc.dram_tensor("ident", [128, 128], bf16, kind="ExternalInput").ap()
    out_d = nc.dram_tensor("out", [S, D], bf16, kind="ExternalOutput").ap()

    from contextlib import ExitStack
    with tile.TileContext(nc) as tc, ExitStack() as ctx:
        const = ctx.enter_context(tc.tile_pool(name="const", bufs=1))
        persist = ctx.enter_context(tc.tile_pool(name="persist", bufs=1))
        xpool = ctx.enter_context(tc.tile_pool(name="xpool", bufs=2))
        scratch = ctx.enter_context(tc.tile_pool(name="scratch", bufs=2))
        small = ctx.enter_context(tc.tile_pool(name="small", bufs=2))
        epool = ctx.enter_context(tc.tile_pool(name="epool", bufs=5))
        accpool = ctx.enter_context(tc.tile_pool(name="accpool", bufs=2))
        qtpool = ctx.enter_context(tc.tile_pool(name="qtpool", bufs=2))
        otpool = ctx.enter_context(tc.tile_pool(name="otpool", bufs=2))
        opool = ctx.enter_context(tc.tile_pool(name="opool", bufs=2))
        psum = ctx.enter_context(tc.tile_pool(name="psum", bufs=4, space="PSUM"))

        # ---- critical path to first matmul: first x chunk + first wq chunk --
        # wq is split into 16 chunk tiles of 2 dt each so the first matmul
        # only depends on chunk 0 (per-tile dependency granularity).
        wq_r = wqkv_d.rearrange("(t p) e -> p t e", p=128)
        wq_t = [persist.tile([128, 2, 768], bf16, name=f"wq{c}")
                for c in range(DT // 2)]
        # x tiles for the dt-major prologue group (st 0-2) are split into
        # per-chunk tiles and interleaved with the wq chunks in DMA issue
        # order, matching the dt-major consumption pattern.
        XS0_CHUNKS = [(0, 4), (4, 8), (8, 16), (16, 24), (24, 32)]
        xsg_t = {}
        for g in range(3):
            xsg_t[g] = [xpool.tile([128, c1 - c0, 128], bf16,
                                   name=f"xs{g}_{i}", bufs=1)
                        for i, (c0, c1) in enumerate(XS0_CHUNKS)]
        nc.sync.dma_start(out=xsg_t[0][0], in_=xt_d[0, :, 0:4, :])
        nc.sync.dma_start(out=wq_t[0], in_=wq_r[:, 0:2, :])
        nc.sync.dma_start(out=xsg_t[1][0], in_=xt_d[1, :, 0:4, :])
        nc.sync.dma_start(out=xsg_t[2][0], in_=xt_d[2, :, 0:4, :])
        nc.sync.dma_start(out=wq_t[1], in_=wq_r[:, 2:4, :])
        for i, (c0, c1) in enumerate(XS0_CHUNKS):
            if i > 0:
                for g in range(3):
                    nc.sync.dma_start(out=xsg_t[g][i], in_=xt_d[g, :, c0:c1, :])
            w0, w1 = (2, 4) if i == 1 else ((4 * i - 4, 4 * i) if i >= 2 else (0, 0))
            for c in range(w0, w1):
                nc.sync.dma_start(out=wq_t[c], in_=wq_r[:, 2 * c:2 * c + 2, :])
        # pre-issue xs3/xs4 so their data lands right as the prologue group
        # finishes (before the rope tables and wo, which are needed later).
        xs_pre = {}
        for st0 in (3, 4):
            xs_p = xpool.tile([128, DT, 128], bf16, name="xs")
            nc.sync.dma_start(out=xs_p, in_=xt_d[st0])
            xs_pre[st0] = xs_p

        # ---- constants / persistent tensors ----
        ccd_sb = const.tile([128, ST, 128], f16)
        nc.sync.dma_start(out=ccd_sb, in_=ccd_d.rearrange("(t p) h -> p t h", p=128))
        ssd_sb = const.tile([128, ST, 128], f16)
        nc.sync.dma_start(out=ssd_sb, in_=ssd_d.rearrange("(t p) h -> p t h", p=128))
        mask_sb = const.tile([128, 128], bf16)
        nc.sync.dma_start(out=mask_sb, in_=mask_d)
        ident_sb = const.tile([128, 128], bf16)
        nc.sync.dma_start(out=ident_sb, in_=ident_d)
        onesm_sb = const.tile([128, 128], bf16)
        nc.vector.memset(onesm_sb, 1.0)

        # warmup matmuls during the initial DMA wait: ~4us of PE activity
        # trips the HAM clock gate to 8/8 before the real matmuls start
        # (otherwise the first ~3.4us of stage 1 runs at 1.2 GHz).
        for w in range(40):
            warm_ps = psum.tile([128, 128], f32, tag="pc", bufs=3, name="warm")
            nc.tensor.matmul(warm_ps, onesm_sb, onesm_sb, start=True, stop=True)

        # stage-3 weights: needed from the first wout units (~150us in); per-h
        # chunk tiles so the first unit (H_ORDER starts at h=2) doesn't wait
        # for the full 4.2 MB.
        wo_r = wo_d.rearrange("(h p) e -> h p e", p=128)
        wo_t = [persist.tile([128, D], bf16, name=f"wo{h}") for h in range(HG)]
        for h in (2, 3, 0, 1):
            nc.sync.dma_start(out=wo_t[h], in_=wo_r[h])

        kT_sb = persist.tile([128, S], bf16)       # [hd, s]
        v_sb = persist.tile([128, ST, 128], bf16)  # [s_local, s_tile, hd]
        # rolling per-qc buffers (written by 4 s-tiles / one att block,
        # consumed one phase later)
        qt_roll = {}   # qc -> [128, HG, 512] bf16
        ot_roll = {}   # qc -> [128, HG, 512] bf16

        # ================= per-s-tile pieces =================
        state = {}  # st -> (q_ps, kv_ps)

        def xs_ap_for(st):
            if st < 3:
                chunks = []
                for i, (c0, c1) in enumerate(XS0_CHUNKS):
                    for d in range(c1 - c0):
                        chunks.append((xsg_t[st][i], d))
                return lambda dt_i: chunks[dt_i][0][:, chunks[dt_i][1], :]
            if st in xs_pre:
                xs = xs_pre[st]
            else:
                xs = xpool.tile([128, DT, 128], bf16, name="xs")
                nc.sync.dma_start(out=xs, in_=xt_d[st])
            return lambda dt_i: xs[:, dt_i, :]

        def mm_group():
            """dt-major qkv matmuls for s-tiles 0-2 together: cuts the
            wq-chunk consumption rate to a third so the PE never outruns the
            wq DMA, and banks ~31us of PE work before the first postproc
            chains need to finish."""
            aps = [xs_ap_for(st) for st in range(3)]
            qps = [psum.tile([128, 512], f32, tag="pa", bufs=3, name=f"q_ps{g}")
                   for g in range(3)]
            kvtag = ["pd", "pd", "pc"]
            kvps = [psum.tile([128, 512], f32, tag=kvtag[g],
                              bufs=(2 if g < 2 else 3), name=f"kv_ps{g}")
                    for g in range(3)]
            for dt_i in range(DT):
                wq_c = wq_t[dt_i // 2][:, dt_i % 2, :]
                for g in range(3):
                    nc.tensor.matmul(qps[g], aps[g](dt_i), wq_c[:, 0:512],
                                     start=(dt_i == 0), stop=(dt_i == DT - 1))
                    nc.tensor.matmul(kvps[g][:, 0:256], aps[g](dt_i),
                                     wq_c[:, 512:768],
                                     start=(dt_i == 0), stop=(dt_i == DT - 1))
            for g in range(3):
                state[g] = (qps[g], kvps[g])

        def mm_tile(st):
            ap = xs_ap_for(st)
            q_ps = psum.tile([128, 512], f32, tag="pa", bufs=3, name="q_ps")
            kv_ps = psum.tile([128, 512], f32, tag="pd", bufs=2, name="kv_ps")
            for dt_i in range(DT):
                wq_c = wq_t[dt_i // 2][:, dt_i % 2, :]
                nc.tensor.matmul(q_ps, ap(dt_i), wq_c[:, 0:512],
                                 start=(dt_i == 0), stop=(dt_i == DT - 1))
                nc.tensor.matmul(kv_ps[:, 0:256], ap(dt_i), wq_c[:, 512:768],
                                 start=(dt_i == 0), stop=(dt_i == DT - 1))
            state[st] = (q_ps, kv_ps)

        fins = {}  # st -> (qfin, kfin)
        casts = {}  # st -> (qb, kb)

        def post_cast(st):
            """Cast q/k out of PSUM to f16 SBUF (frees the pa/pd psum slots
            within ~1us so later tiles' matmuls can start). Emitted right
            after the tile's matmuls; in the prologue the casts of several
            tiles are emitted ahead of the math parts so the DVE frees all
            accumulators before grinding the serial rope chains."""
            q_ps, kv_ps = state.pop(st)
            kb = small.tile([128, 128], f16, bufs=3)
            nc.vector.tensor_copy(out=kb, in_=kv_ps[:, 0:128])
            nc.vector.tensor_copy(out=v_sb[:, st, :], in_=kv_ps[:, 128:256])
            qb = scratch.tile([128, 512], f16, bufs=3)
            nc.vector.tensor_copy(out=qb, in_=q_ps)
            casts[st] = (qb, kb)

        def post_math(st):
            """Rope + rmsnorm stats + final bf16 q/k tiles (all f16 DVE)."""
            qb, kb = casts.pop(st)

            # rope (sumsq comes from the rope'd values -- rope is a per-pair
            # rotation so the head norms are unchanged; keeps Square off ACT
            # so its table cache only holds {Copy, Exp}).
            qb4 = qb.rearrange("p (h e) -> p h e", h=HG)
            rot_q = scratch.tile([128, HG, 64, 2], f16)
            nc.vector.tensor_copy(
                out=rot_q,
                in_=qb.rearrange("p (h r two) -> p h r two", h=HG, two=2)[
                    :, :, :, ::-1])
            cc_b = ccd_sb[:, st, :].unsqueeze(1).broadcast_to((128, HG, 128))
            ss_b = ssd_sb[:, st, :].unsqueeze(1).broadcast_to((128, HG, 128))
            qcc = scratch.tile([128, HG, 128], f16)
            nc.vector.tensor_mul(qcc, qb4, cc_b)
            qss = scratch.tile([128, HG, 128], f16)
            nc.vector.tensor_mul(qss, rot_q.rearrange("p h r two -> p h (r two)"),
                                 ss_b)
            qrope = scratch.tile([128, HG, 128], f16)
            nc.vector.tensor_add(qrope, qcc, qss)

            # rope k
            rot_k = small.tile([128, 64, 2], f16)
            nc.vector.tensor_copy(
                out=rot_k,
                in_=kb.rearrange("p (r two) -> p r two", two=2)[:, :, ::-1])
            kcc = small.tile([128, 128], f16)
            nc.vector.tensor_mul(kcc, kb, ccd_sb[:, st, :])
            kss = small.tile([128, 128], f16)
            nc.vector.tensor_mul(kss, rot_k.rearrange("p r two -> p (r two)"),
                                 ssd_sb[:, st, :])
            krope = small.tile([128, 128], f16)
            nc.vector.tensor_add(krope, kcc, kss)

            # sum of squares per head from the rope'd values; the squares
            # overwrite qcc/kcc (dead after the rope adds) to save SBUF
            nc.vector.tensor_mul(qcc, qrope, qrope)
            ssq = small.tile([128, 5], f32)
            nc.vector.tensor_reduce(
                out=ssq[:, 0:4].rearrange("p (h one) -> p h one", one=1),
                in_=qcc, axis=mybir.AxisListType.X, op=mybir.AluOpType.add)
            nc.vector.tensor_mul(kcc, krope, krope)
            nc.vector.tensor_reduce(
                out=ssq[:, 4:5], in_=kcc,
                axis=mybir.AxisListType.X, op=mybir.AluOpType.add)
            # rstd = 1/sqrt(ssq*scale + eps) via DVE fast-inverse-sqrt (magic
            # seed + one Newton step, rel err ~2e-3). Keeps Sqrt off ACT so
            # its table cache only ever holds {Copy, Exp} -- no reloads.
            # q cols get the 1/sqrt(HD) score scale folded in.
            x5 = small.tile([128, 5], f32)
            nc.vector.tensor_scalar_add(x5[:, 0:4], ssq[:, 0:4],
                                        float(HD * EPS))
            nc.vector.tensor_scalar(out=x5[:, 4:5], in0=ssq[:, 4:5],
                                    scalar1=1.0 / HD, scalar2=float(EPS),
                                    op0=mybir.AluOpType.mult,
                                    op1=mybir.AluOpType.add)
            xi = x5.bitcast(i32)
            t5 = small.tile([128, 5], i32)
            nc.vector.tensor_scalar(out=t5, in0=xi, scalar1=1, scalar2=None,
                                    op0=mybir.AluOpType.arith_shift_right)
            # y0i = MAGIC - t  ==  (t ^ -1) + (MAGIC + 1)
            nt5 = small.tile([128, 5], i32)
            nc.vector.tensor_scalar(out=nt5, in0=t5, scalar1=-1, scalar2=None,
                                    op0=mybir.AluOpType.bitwise_xor)
            y0i = small.tile([128, 5], i32)
            nc.vector.tensor_scalar_add(y0i, nt5, 0x5f375a86 + 1)
            y0 = y0i.bitcast(f32)
            a5 = small.tile([128, 5], f32)
            nc.vector.tensor_mul(a5, x5, y0)
            b5 = small.tile([128, 5], f32)
            nc.vector.tensor_mul(b5, a5, y0)            # x*y0^2
            c5 = small.tile([128, 5], f32)
            nc.vector.tensor_scalar(out=c5, in0=b5, scalar1=-0.5, scalar2=1.5,
                                    op0=mybir.AluOpType.mult,
                                    op1=mybir.AluOpType.add)
            rstd = small.tile([128, 5], f32)
            nc.vector.tensor_mul(rstd, y0, c5)

            qfin = scratch.tile([128, HG, 128], bf16, bufs=4)
            for hh in range(HG):
                nc.vector.tensor_scalar_mul(qfin[:, hh, :], qrope[:, hh, :],
                                            rstd[:, hh:hh + 1])
            kfin = small.tile([128, 128], bf16, bufs=4)
            nc.vector.tensor_scalar_mul(kfin, krope, rstd[:, 4:5])
            fins[st] = (qfin, kfin)

        def post_transp(st):
            """PE transposes of q/k into [hd, s]; lagged one s-tile so the
            post_calc chain hides under the next tile's matmuls."""
            qfin, kfin = fins.pop(st)
            qc, sl = st // 4, st % 4
            if sl == 0:
                qt_roll[qc] = qtpool.tile([128, HG, 512], bf16, name="qt")
            for hh in range(HG):
                tq_ps = psum.tile([128, 128], bf16, tag="pc", bufs=3)
                nc.tensor.transpose(tq_ps, qfin[:, hh, :], ident_sb)
                nc.scalar.copy(out=qt_roll[qc][:, hh, sl * 128:(sl + 1) * 128],
                               in_=tq_ps)
            tk_ps = psum.tile([128, 128], bf16, tag="pc", bufs=3)
            nc.tensor.transpose(tk_ps, kfin, ident_sb)
            nc.scalar.copy(out=kT_sb[:, st * 128:(st + 1) * 128], in_=tk_ps)

        # ================= out-projection units ====================
        # one unit = one [128,512] ec-chunk of y for one s-tile: 4 matmuls
        # accumulating over this core's 4 heads in a single psum bank, then
        # one copy to SBUF (bf16) and per-half (or per-unit for the final
        # qc) DMA out. Units are spliced between attention kt-iterations to
        # fill the PE idle left by the ACT-bound exp chain.
        H_ORDER = (2, 3, 0, 1)  # h2 first: hp=1 normalizes first (hp order 1,0)

        def wout_units(qc):
            oT = ot_roll[qc]
            for sl in range(4):
                st = 4 * qc + sl
                out_sb = opool.tile([128, D], bf16, name="out_sb")
                for half in range(2):
                    for i in range(4):
                        ec = half * 4 + i
                        o_ps = psum.tile([128, 512], f32, tag="pd", bufs=2,
                                         name="o_ps")
                        for h in H_ORDER:
                            nc.tensor.matmul(
                                o_ps,
                                oT[:, h, sl * 128:(sl + 1) * 128],
                                wo_t[h][:, ec * 512:(ec + 1) * 512],
                                start=(h == H_ORDER[0]), stop=(h == H_ORDER[-1]))
                        if i % 2 == 0:
                            nc.scalar.copy(
                                out=out_sb[:, ec * 512:(ec + 1) * 512], in_=o_ps)
                        else:
                            nc.vector.tensor_copy(
                                out=out_sb[:, ec * 512:(ec + 1) * 512], in_=o_ps)
                        if qc == QC - 1:
                            nc.sync.dma_start(
                                out=out_d[st * 128:(st + 1) * 128,
                                          ec * 512:(ec + 1) * 512],
                                in_=out_sb[:, ec * 512:(ec + 1) * 512])
                        yield
                    if qc != QC - 1:
                        nc.sync.dma_start(
                            out=out_d[st * 128:(st + 1) * 128,
                                      half * 2048:(half + 1) * 2048],
                            in_=out_sb[:, half * 2048:(half + 1) * 2048])

        def drain(gen, n=10 ** 9):
            if gen is None:
                return True
            for _ in range(n):
                try:
                    next(gen)
                except StopIteration:
                    return True
            return False

        # ================= attention ====================
        def att(qc, splice=None):
            ot_roll[qc] = otpool.tile([128, HG, 512], bf16, name="ot")
            qt = qt_roll.pop(qc)
            it = 0
            for hp in (1, 0):
                hh0 = 2 * hp
                pv0 = psum.tile([128, 512], f32, tag="pa", bufs=3, name="pv0")
                pv1 = psum.tile([128, 512], f32, tag="pa", bufs=3, name="pv1")
                # softmax denominator: accumulate exp tiles on DVE (bf16 =
                # 2x DVE rate), partition-reduce at the end via one
                # ones-matmul -> [128 identical rows, 512] broadcast.
                ea0 = accpool.tile([128, 512], bf16, name="ea0")
                ea1 = accpool.tile([128, 512], bf16, name="ea1")
                pvs, eas = [pv0, pv1], [ea0, ea1]
                n_kt = 4 * qc + 4
                for kt in range(n_kt):
                    j = kt - 4 * qc
                    off = 0 if j < 0 else 128 * j
                    exs = []
                    for hi in range(2):
                        h = hh0 + hi
                        sc_ps = psum.tile([128, 512], f32, tag="pc", bufs=3,
                                          name="sc")
                        nc.tensor.matmul(
                            sc_ps[:, off:512],
                            kT_sb[:, kt * 128:(kt + 1) * 128],
                            qt[:, h, off:512],
                            start=True, stop=True)
                        ex = epool.tile([128, 512], bf16, name=f"ex_{hi}")
                        nc.scalar.activation(out=ex[:, off:512],
                                             in_=sc_ps[:, off:512], func=AF.Exp)
                        if j >= 0:
                            nc.vector.tensor_mul(ex[:, off:off + 128],
                                                 ex[:, off:off + 128], mask_sb)
                        if kt == 0:
                            nc.vector.tensor_copy(out=eas[hi], in_=ex)
                        else:
                            nc.vector.tensor_add(eas[hi][:, off:512],
                                                 eas[hi][:, off:512],
                                                 ex[:, off:512])
                        exs.append(ex)
                    for hi in range(2):
                        nc.tensor.matmul(pvs[hi][:, off:512], v_sb[:, kt, :],
                                         exs[hi][:, off:512],
                                         start=(kt == 0), stop=(kt == n_kt - 1))
                    it += 1
                    if splice is not None and it % 3 == 0:
                        drain(splice, 1)
                for hi in range(2):
                    h = hh0 + hi
                    den_ps = psum.tile([128, 512], f32, tag="pd", bufs=2,
                                       name="den")
                    nc.tensor.matmul(den_ps, onesm_sb, eas[hi],
                                     start=True, stop=True)
                    rden = scratch.tile([128, 512], f32, tag="rden")
                    nc.vector.reciprocal_approx_fast(out=rden, in_=den_ps)
                    nc.vector.tensor_mul(ot_roll[qc][:, h, :], pvs[hi], rden)

        # ================= fused schedule ====================
        # post_calc right after each tile's matmuls (frees PSUM accumulators
        # before any att block -- holding them across one deadlocks the slot
        # rotation); transposes lag one tile; wout units spliced into the
        # following att block to fill the exp-bound PE idle.
        # mm3/mm4 are emitted BEFORE the prologue post_calcs: the scheduler
        # builds a static per-engine order from emission priority, and the PE
        # must have matmul work queued ahead of the transposes while the
        # three serial postproc chains drain on the DVE.
        def post_calc(st):
            post_cast(st)
            post_math(st)

        mm_group(); mm_tile(3); mm_tile(4)
        post_cast(0); post_cast(1); post_cast(2)
        post_math(0); post_cast(3)
        post_math(1); post_cast(4)
        post_math(2); post_math(3); post_math(4)
        post_transp(0); post_transp(1); post_transp(2); post_transp(3)
        att(0)
        post_transp(4)
        mm_tile(5); post_calc(5)
        mm_tile(6); post_calc(6); post_transp(5)
        mm_tile(7); post_calc(7); post_transp(6)
        mm_tile(8); post_calc(8); post_transp(7)
        w0 = wout_units(0)
        att(1, splice=w0)
        drain(w0)
        post_transp(8)
        mm_tile(9); post_calc(9)
        mm_tile(10); post_calc(10); post_transp(9)
        mm_tile(11); post_calc(11); post_transp(10)
        mm_tile(12); post_calc(12); post_transp(11)
        w1 = wout_units(1)
        att(2, splice=w1)
        drain(w1)
        post_transp(12)
        mm_tile(13); post_calc(13)
        mm_tile(14); post_calc(14); post_transp(13)
        mm_tile(15); post_calc(15); post_transp(14)
        post_transp(15)
        w2 = wout_units(2)
        drain(w2, 5)
        att(3, splice=w2)
        drain(w2)
        w3 = wout_units(3)
        drain(w3)

    nc.compile()
    return nc


def _host_prep(x, w_qkv, w_out, q_ln_w, k_ln_w):
    """Build per-core input maps (host-side shard + transform)."""
    bf = np.dtype("bfloat16") if hasattr(np, "bfloat16") else None
    import ml_dtypes
    bf16 = ml_dtypes.bfloat16

    x2 = np.asarray(x, np.float32).reshape(S, D)
    # x tiles [st, d_local, d_tile, s_local] so each s-tile DMA is contiguous
    xt = np.ascontiguousarray(
        x2.reshape(ST, 128, DT, 128).transpose(0, 3, 2, 1)).astype(bf16)

    # rope tables (duplicated cos / sign-baked sin, interleaved layout)
    freqs = 1.0 / (THETA ** (np.arange(0, HD, 2, dtype=np.float64) / HD))
    ang = np.arange(S, dtype=np.float64)[:, None] * freqs[None, :]
    cos = np.cos(ang).astype(np.float32)
    sin = np.sin(ang).astype(np.float32)
    ccd = np.repeat(cos, 2, axis=1).astype(np.float16)    # [S, 128]
    ssd = np.stack([-sin, sin], axis=-1).reshape(S, HD).astype(np.float16)

    kq = np.arange(128)
    dmask = (kq[:, None] <= kq[None, :]).astype(bf16)     # [k, q]
    ident = np.eye(128, dtype=bf16)

    wq = np.asarray(w_qkv, np.float32)
    wo = np.asarray(w_out, np.float32)
    qw = np.asarray(q_ln_w, np.float32)
    kw = np.asarray(k_ln_w, np.float32)

    in_maps = []
    for g in range(N_CORES):
        wq_g = wq[512 * g:512 * (g + 1), :].reshape(HG, HD, D) * qw[None, :, None]
        wk_g = wq[D + 128 * g:D + 128 * (g + 1), :] * kw[:, None]
        wv_g = wq[D + G * HD + 128 * g:D + G * HD + 128 * (g + 1), :]
        wqkv_g = np.concatenate([wq_g.reshape(512, D), wk_g, wv_g], axis=0)
        wqkvT_g = np.ascontiguousarray(wqkv_g.T).astype(bf16)     # [D, 768]
        woT_g = np.ascontiguousarray(wo[:, 512 * g:512 * (g + 1)].T).astype(bf16)
        in_maps.append({
            "xt": xt,
            "wqkvT": wqkvT_g,
            "woT": woT_g,
            "ccd": ccd,
            "ssd": ssd,
            "dmask": dmask,
            "ident": ident,
        })
    return in_maps


_CACHE = {}


def _get_compiled():
    if "nc" not in _CACHE:
        _ensure_ntff_hook()
        _CACHE["nc"] = _build_nc()
    return _CACHE["nc"]


def run_sharded(x, w_qkv, w_out, q_ln_w, k_ln_w, trace=False):
    from concourse.bass_utils import run_bass_kernel_spmd
    nc = _get_compiled()
    in_maps = _host_prep(x, w_qkv, w_out, q_ln_w, k_ln_w)
    res = run_bass_kernel_spmd(nc, in_maps, core_ids=list(range(N_CORES)),
                               trace=trace)
    acc = np.zeros((S, D), np.float32)
    for i in range(N_CORES):
        acc += np.asarray(res.results[i]["out"], np.float32)
    return acc.reshape(1, S, D), res


def kernel(x, w_qkv, w_out, q_ln_w, k_ln_w):
    out, _ = run_sharded(x, w_qkv, w_out, q_ln_w, k_ln_w, trace=False)
    return out



# revision 2
# speedup vs baseline: 1.0311x; 1.0311x over previous
"""Trainium2 Bass kernel for GQA multi-head attention block (nn_MHA_68831145886222).

Computation (reference):
  qkv = x @ w_qkv.T ; split q[32 heads],k[8],v[8] (HD=128)
  q,k = rmsnorm(head_dim) -> rope(interleaved, theta=1e6)
  out = causal GQA attention (4 q heads per kv head)
  y   = (attn out) @ w_out.T

Sharding: tensor-parallel by kv-head group. Core g of 8 owns q heads
4g..4g+3 and kv head g (columns of the qkv projection), plus the matching
512 input rows of w_out. Each core computes a partial y [2048,4096]; the
host sums the 8 partials.

Device-side layout choices per core:
  stage 1 (qkv proj):  stationary = x^T tiles [128 d, 128 s] (bf16),
                       moving = w_qkv^T slices -> qkv in natural [s, e] psum
  postproc: rmsnorm stats via DVE Square+reduce; rope via pair-swap copy +
            two muls + add (tables host-precomputed); per-head rstd apply
            (score scale 1/sqrt(HD) and ln weights folded in); PE-transpose
            q,k to [hd, s]; v stays natural [s, hd].
  attention: scoresT [k, q] = kT-tile.T @ qT (exact causal via restricted
             moving dim); exp on ACT; diagonal 128x128 blocks masked by a
             0/1 mask mul; PV and the softmax denominator (ones-matmul)
             accumulate over k-tiles in PSUM; normalize after PV.
  stage 3 (out proj): stationary = attnT [128 hd, 128 s], moving = w_out^T
             slices; accumulate over the core's 4 heads; fp32 partial out.

Scheduling: the PE is the bottleneck engine (~92% busy in the 396us
baseline); the remaining idle was concentrated at att(0) (ACT exp-bound,
nothing to splice -> HAM re-throttle) and at attention block/hp
boundaries. This version splices qkv tile 5's matmuls INTO att(0), moves
all splice drains between the score and PV matmuls (where the PE would
otherwise wait on ACT), pre-drains wout units at block/hp/den boundaries,
and re-tags PSUM so the spliced accumulators get dedicated banks
(pa/pb/pc/pd all bufs=2 = 8 banks) without slot-rotation deadlocks.
"""

import os
import sys
import types

import numpy as np

H = 32
G = 8
HD = 128
S = 2048
D = 4096
HG = H // G  # q heads per kv head = 4
EPS = 1e-5
THETA = 1e6
N_CORES = 8
ST = S // 128  # 16 s-tiles
DT = D // 128  # 32 d-tiles
QC = 4  # q chunks of 512
EC = 8  # e chunks of 512 in final matmul


def _ensure_ntff_hook():
    """Register the axon NTFF profile hook if the image's antenv lacks it,
    so run_bass_kernel_spmd(trace=True) can return exec_time_ns."""
    try:
        from antenv.axon_hooks import get_axon_ntff_profile_hook  # noqa: F401
        return
    except ImportError:
        pass
    try:
        import antenv
        mod = types.ModuleType("antenv.axon_hooks")
        _h = [None]
        mod.set_axon_ntff_profile_hook = lambda h: _h.__setitem__(0, h)
        mod.get_axon_ntff_profile_hook = lambda: _h[0]
        sys.modules["antenv.axon_hooks"] = mod
        antenv.axon_hooks = mod
        from trn_agent_boot.trn_boot import _ntff_profile_via_ctypes
        so = "/opt/axon/libaxon_pjrt.so"
        if os.path.exists(so):
            mod.set_axon_ntff_profile_hook(_ntff_profile_via_ctypes(so))
    except Exception:
        pass


def _build_nc():
    import concourse.bass as bass  # noqa: F401
    import concourse.tile as tile
    from concourse import bacc, mybir

    bf16 = mybir.dt.bfloat16
    f16 = mybir.dt.float16
    f32 = mybir.dt.float32
    i32 = mybir.dt.int32
    AF = mybir.ActivationFunctionType

    nc = bacc.Bacc("TRN2", target_bir_lowering=False, debug=False,
                   num_devices=N_CORES)

    # ---- DRAM I/O ----
    xt_d = nc.dram_tensor("xt", [ST, 128, DT, 128], bf16, kind="ExternalInput").ap()
    wqkv_d = nc.dram_tensor("wqkvT", [D, 768], bf16, kind="ExternalInput").ap()
    wo_d = nc.dram_tensor("woT", [512, D], bf16, kind="ExternalInput").ap()
    ccd_d = nc.dram_tensor("ccd", [S, 128], f16, kind="ExternalInput").ap()
    ssd_d = nc.dram_tensor("ssd", [S, 128], f16, kind="ExternalInput").ap()
    mask_d = nc.dram_tensor("dmask", [128, 128], bf16, kind="ExternalInput").ap()
    ident_d = nc.dram_tensor("ident", [128, 128], bf16, kind="ExternalInput").ap()
    out_d = nc.dram_tensor("out", [S, D], bf16, kind="ExternalOutput").ap()

    from contextlib import ExitStack
    with tile.TileContext(nc) as tc, ExitStack() as ctx:
        const = ctx.enter_context(tc.tile_pool(name="const", bufs=1))
        persist = ctx.enter_context(tc.tile_pool(name="persist", bufs=1))
        xpool = ctx.enter_context(tc.tile_pool(name="xpool", bufs=2))
        scratch = ctx.enter_context(tc.tile_pool(name="scratch", bufs=2))
        small = ctx.enter_context(tc.tile_pool(name="small", bufs=2))
        epool = ctx.enter_context(tc.tile_pool(name="epool", bufs=5))
        accpool = ctx.enter_context(tc.tile_pool(name="accpool", bufs=2))
        qtpool = ctx.enter_context(tc.tile_pool(name="qtpool", bufs=2))
        otpool = ctx.enter_context(tc.tile_pool(name="otpool", bufs=2))
        opool = ctx.enter_context(tc.tile_pool(name="opool", bufs=2))
        psum = ctx.enter_context(tc.tile_pool(name="psum", bufs=2, space="PSUM"))

        # ---- critical path to first matmul: first x chunk + first wq chunk --
        # wq is split into 16 chunk tiles of 2 dt each so the first matmul
        # only depends on chunk 0 (per-tile dependency granularity).
        wq_r = wqkv_d.rearrange("(t p) e -> p t e", p=128)
        wq_t = [persist.tile([128, 2, 768], bf16, name=f"wq{c}")
                for c in range(DT // 2)]
        # x tiles for the dt-major prologue group (st 0-2) are split into
        # per-chunk tiles and interleaved with the wq chunks in DMA issue
        # order, matching the dt-major consumption pattern.
        XS0_CHUNKS = [(0, 4), (4, 8), (8, 16), (16, 24), (24, 32)]
        xsg_t = {}
        for g in range(3):
            xsg_t[g] = [xpool.tile([128, c1 - c0, 128], bf16,
                                   name=f"xs{g}_{i}", bufs=1)
                        for i, (c0, c1) in enumerate(XS0_CHUNKS)]
        nc.sync.dma_start(out=xsg_t[0][0], in_=xt_d[0, :, 0:4, :])
        nc.sync.dma_start(out=wq_t[0], in_=wq_r[:, 0:2, :])
        nc.sync.dma_start(out=xsg_t[1][0], in_=xt_d[1, :, 0:4, :])
        nc.sync.dma_start(out=xsg_t[2][0], in_=xt_d[2, :, 0:4, :])
        nc.sync.dma_start(out=wq_t[1], in_=wq_r[:, 2:4, :])
        for i, (c0, c1) in enumerate(XS0_CHUNKS):
            if i > 0:
                for g in range(3):
                    nc.sync.dma_start(out=xsg_t[g][i], in_=xt_d[g, :, c0:c1, :])
            w0, w1 = (2, 4) if i == 1 else ((4 * i - 4, 4 * i) if i >= 2 else (0, 0))
            for c in range(w0, w1):
                nc.sync.dma_start(out=wq_t[c], in_=wq_r[:, 2 * c:2 * c + 2, :])
        # pre-issue xs3/xs4 so their data lands right as the prologue group
        # finishes (before the rope tables and wo, which are needed later).
        xs_pre = {}
        for st0 in (3, 4):
            xs_p = xpool.tile([128, DT, 128], bf16, name="xs")
            nc.sync.dma_start(out=xs_p, in_=xt_d[st0])
            xs_pre[st0] = xs_p

        # ---- constants / persistent tensors ----
        ccd_sb = const.tile([128, ST, 128], f16)
        nc.sync.dma_start(out=ccd_sb, in_=ccd_d.rearrange("(t p) h -> p t h", p=128))
        ssd_sb = const.tile([128, ST, 128], f16)
        nc.sync.dma_start(out=ssd_sb, in_=ssd_d.rearrange("(t p) h -> p t h", p=128))
        mask_sb = const.tile([128, 128], bf16)
        nc.sync.dma_start(out=mask_sb, in_=mask_d)
        ident_sb = const.tile([128, 128], bf16)
        nc.sync.dma_start(out=ident_sb, in_=ident_d)
        onesm_sb = const.tile([128, 128], bf16)
        nc.vector.memset(onesm_sb, 1.0)

        # xs5 pre-issue after the tables (tile 5 is consumed early -- spliced
        # into att(0) -- but the rope tables are needed earlier still).
        xs5_p = xpool.tile([128, DT, 128], bf16, name="xs")
        nc.sync.dma_start(out=xs5_p, in_=xt_d[5])
        xs_pre[5] = xs5_p

        # warmup matmuls during the initial DMA wait: ~4us of PE activity
        # trips the HAM clock gate to 8/8 before the real matmuls start
        # (otherwise the first ~3.4us of stage 1 runs at 1.2 GHz).
        for w in range(40):
            warm_ps = psum.tile([128, 128], f32, tag="pb", bufs=2, name="warm")
            nc.tensor.matmul(warm_ps, onesm_sb, onesm_sb, start=True, stop=True)

        # stage-3 weights: needed from the first wout units (~150us in); per-h
        # chunk tiles so the first unit (H_ORDER starts at h=2) doesn't wait
        # for the full 4.2 MB.
        wo_r = wo_d.rearrange("(h p) e -> h p e", p=128)
        wo_t = [persist.tile([128, D], bf16, name=f"wo{h}") for h in range(HG)]
        for h in (2, 3, 0, 1):
            nc.sync.dma_start(out=wo_t[h], in_=wo_r[h])

        kT_sb = persist.tile([128, S], bf16)       # [hd, s]
        v_sb = persist.tile([128, ST, 128], bf16)  # [s_local, s_tile, hd]
        # rolling per-qc buffers (written by 4 s-tiles / one att block,
        # consumed one phase later)
        qt_roll = {}   # qc -> [128, HG, 512] bf16
        ot_roll = {}   # qc -> [128, HG, 512] bf16

        # ================= per-s-tile pieces =================
        state = {}  # st -> (q_ps, kv_ps)

        def xs_ap_for(st):
            if st < 3:
                chunks = []
                for i, (c0, c1) in enumerate(XS0_CHUNKS):
                    for d in range(c1 - c0):
                        chunks.append((xsg_t[st][i], d))
                return lambda dt_i: chunks[dt_i][0][:, chunks[dt_i][1], :]
            if st in xs_pre:
                xs = xs_pre[st]
            else:
                xs = xpool.tile([128, DT, 128], bf16, name="xs")
                nc.sync.dma_start(out=xs, in_=xt_d[st])
            return lambda dt_i: xs[:, dt_i, :]

        def mm_group():
            """dt-major qkv matmuls for s-tiles 0-2 together: cuts the
            wq-chunk consumption rate to a third so the PE never outruns the
            wq DMA, and banks ~31us of PE work before the first postproc
            chains need to finish."""
            aps = [xs_ap_for(st) for st in range(3)]
            qtags = ["pa", "pa", "pb"]
            qps = [psum.tile([128, 512], f32, tag=qtags[g], bufs=2,
                             name=f"q_ps{g}")
                   for g in range(3)]
            kvtags = ["pd", "pd", "pb"]
            kvps = [psum.tile([128, 512], f32, tag=kvtags[g], bufs=2,
                              name=f"kv_ps{g}")
                    for g in range(3)]
            for dt_i in range(DT):
                wq_c = wq_t[dt_i // 2][:, dt_i % 2, :]
                for g in range(3):
                    nc.tensor.matmul(qps[g], aps[g](dt_i), wq_c[:, 0:512],
                                     start=(dt_i == 0), stop=(dt_i == DT - 1))
                    nc.tensor.matmul(kvps[g][:, 0:256], aps[g](dt_i),
                                     wq_c[:, 512:768],
                                     start=(dt_i == 0), stop=(dt_i == DT - 1))
            for g in range(3):
                state[g] = (qps[g], kvps[g])

        def mm_tile(st, qtag="pa", kvtag="pd"):
            ap = xs_ap_for(st)
            q_ps = psum.tile([128, 512], f32, tag=qtag, bufs=2, name="q_ps")
            kv_ps = psum.tile([128, 512], f32, tag=kvtag, bufs=2, name="kv_ps")
            for dt_i in range(DT):
                wq_c = wq_t[dt_i // 2][:, dt_i % 2, :]
                nc.tensor.matmul(q_ps, ap(dt_i), wq_c[:, 0:512],
                                 start=(dt_i == 0), stop=(dt_i == DT - 1))
                nc.tensor.matmul(kv_ps[:, 0:256], ap(dt_i), wq_c[:, 512:768],
                                 start=(dt_i == 0), stop=(dt_i == DT - 1))
            state[st] = (q_ps, kv_ps)

        def mm_tile_gen(st):
            """Generator form of mm_tile for splicing into an att block: the
            accumulators live on the dedicated pb tag so the slot rotation of
            the att block's own psum tiles (pa/pc/pd) can't cycle on them."""
            ap = xs_ap_for(st)
            q_ps = psum.tile([128, 512], f32, tag="pb", bufs=2, name="q_ps")
            kv_ps = psum.tile([128, 512], f32, tag="pb", bufs=2, name="kv_ps")
            state[st] = (q_ps, kv_ps)
            for dt_i in range(DT):
                wq_c = wq_t[dt_i // 2][:, dt_i % 2, :]
                nc.tensor.matmul(q_ps, ap(dt_i), wq_c[:, 0:512],
                                 start=(dt_i == 0), stop=(dt_i == DT - 1))
                nc.tensor.matmul(kv_ps[:, 0:256], ap(dt_i), wq_c[:, 512:768],
                                 start=(dt_i == 0), stop=(dt_i == DT - 1))
                yield

        fins = {}  # st -> (qfin, kfin)
        casts = {}  # st -> (qb, kb)

        def post_cast(st):
            """Cast q/k out of PSUM to f16 SBUF (frees the psum slots
            within ~1us so later tiles' matmuls can start). Emitted right
            after the tile's matmuls; in the prologue the casts of several
            tiles are emitted ahead of the math parts so the DVE frees all
            accumulators before grinding the serial rope chains."""
            q_ps, kv_ps = state.pop(st)
            kb = small.tile([128, 128], f16, bufs=3)
            nc.vector.tensor_copy(out=kb, in_=kv_ps[:, 0:128])
            nc.vector.tensor_copy(out=v_sb[:, st, :], in_=kv_ps[:, 128:256])
            qb = scratch.tile([128, 512], f16, bufs=3)
            nc.vector.tensor_copy(out=qb, in_=q_ps)
            casts[st] = (qb, kb)

        def post_math(st):
            """Rope + rmsnorm stats + final bf16 q/k tiles (all f16 DVE)."""
            qb, kb = casts.pop(st)

            # rope (sumsq comes from the rope'd values -- rope is a per-pair
            # rotation so the head norms are unchanged; keeps Square off ACT
            # so its table cache only holds {Copy, Exp}).
            qb4 = qb.rearrange("p (h e) -> p h e", h=HG)
            rot_q = scratch.tile([128, HG, 64, 2], f16)
            nc.vector.tensor_copy(
                out=rot_q,
                in_=qb.rearrange("p (h r two) -> p h r two", h=HG, two=2)[
                    :, :, :, ::-1])
            cc_b = ccd_sb[:, st, :].unsqueeze(1).broadcast_to((128, HG, 128))
            ss_b = ssd_sb[:, st, :].unsqueeze(1).broadcast_to((128, HG, 128))
            qcc = scratch.tile([128, HG, 128], f16)
            nc.vector.tensor_mul(qcc, qb4, cc_b)
            qss = scratch.tile([128, HG, 128], f16)
            nc.vector.tensor_mul(qss, rot_q.rearrange("p h r two -> p h (r two)"),
                                 ss_b)
            qrope = scratch.tile([128, HG, 128], f16)
            nc.vector.tensor_add(qrope, qcc, qss)

            # rope k
            rot_k = small.tile([128, 64, 2], f16)
            nc.vector.tensor_copy(
                out=rot_k,
                in_=kb.rearrange("p (r two) -> p r two", two=2)[:, :, ::-1])
            kcc = small.tile([128, 128], f16)
            nc.vector.tensor_mul(kcc, kb, ccd_sb[:, st, :])
            kss = small.tile([128, 128], f16)
            nc.vector.tensor_mul(kss, rot_k.rearrange("p r two -> p (r two)"),
                                 ssd_sb[:, st, :])
            krope = small.tile([128, 128], f16)
            nc.vector.tensor_add(krope, kcc, kss)

            # sum of squares per head from the rope'd values; the squares
            # overwrite qcc/kcc (dead after the rope adds) to save SBUF
            nc.vector.tensor_mul(qcc, qrope, qrope)
            ssq = small.tile([128, 5], f32)
            nc.vector.tensor_reduce(
                out=ssq[:, 0:4].rearrange("p (h one) -> p h one", one=1),
                in_=qcc, axis=mybir.AxisListType.X, op=mybir.AluOpType.add)
            nc.vector.tensor_mul(kcc, krope, krope)
            nc.vector.tensor_reduce(
                out=ssq[:, 4:5], in_=kcc,
                axis=mybir.AxisListType.X, op=mybir.AluOpType.add)
            # rstd = 1/sqrt(ssq*scale + eps) via DVE fast-inverse-sqrt (magic
            # seed + one Newton step, rel err ~2e-3). Keeps Sqrt off ACT so
            # its table cache only ever holds {Copy, Exp} -- no reloads.
            # q cols get the 1/sqrt(HD) score scale folded in.
            x5 = small.tile([128, 5], f32)
            nc.vector.tensor_scalar_add(x5[:, 0:4], ssq[:, 0:4],
                                        float(HD * EPS))
            nc.vector.tensor_scalar(out=x5[:, 4:5], in0=ssq[:, 4:5],
                                    scalar1=1.0 / HD, scalar2=float(EPS),
                                    op0=mybir.AluOpType.mult,
                                    op1=mybir.AluOpType.add)
            xi = x5.bitcast(i32)
            t5 = small.tile([128, 5], i32)
            nc.vector.tensor_scalar(out=t5, in0=xi, scalar1=1, scalar2=None,
                                    op0=mybir.AluOpType.arith_shift_right)
            # y0i = MAGIC - t  ==  (t ^ -1) + (MAGIC + 1)
            nt5 = small.tile([128, 5], i32)
            nc.vector.tensor_scalar(out=nt5, in0=t5, scalar1=-1, scalar2=None,
                                    op0=mybir.AluOpType.bitwise_xor)
            y0i = small.tile([128, 5], i32)
            nc.vector.tensor_scalar_add(y0i, nt5, 0x5f375a86 + 1)
            y0 = y0i.bitcast(f32)
            a5 = small.tile([128, 5], f32)
            nc.vector.tensor_mul(a5, x5, y0)
            b5 = small.tile([128, 5], f32)
            nc.vector.tensor_mul(b5, a5, y0)            # x*y0^2
            c5 = small.tile([128, 5], f32)
            nc.vector.tensor_scalar(out=c5, in0=b5, scalar1=-0.5, scalar2=1.5,
                                    op0=mybir.AluOpType.mult,
                                    op1=mybir.AluOpType.add)
            rstd = small.tile([128, 5], f32)
            nc.vector.tensor_mul(rstd, y0, c5)

            qfin = scratch.tile([128, HG, 128], bf16, bufs=4)
            for hh in range(HG):
                nc.vector.tensor_scalar_mul(qfin[:, hh, :], qrope[:, hh, :],
                                            rstd[:, hh:hh + 1])
            kfin = small.tile([128, 128], bf16, bufs=4)
            nc.vector.tensor_scalar_mul(kfin, krope, rstd[:, 4:5])
            fins[st] = (qfin, kfin)

        def post_transp(st):
            """PE transposes of q/k into [hd, s]; lagged one s-tile so the
            post_calc chain hides under the next tile's matmuls."""
            qfin, kfin = fins.pop(st)
            qc, sl = st // 4, st % 4
            if sl == 0:
                qt_roll[qc] = qtpool.tile([128, HG, 512], bf16, name="qt")
            for hh in range(HG):
                tq_ps = psum.tile([128, 128], bf16, tag="pc", bufs=2)
                nc.tensor.transpose(tq_ps, qfin[:, hh, :], ident_sb)
                nc.scalar.copy(out=qt_roll[qc][:, hh, sl * 128:(sl + 1) * 128],
                               in_=tq_ps)
            tk_ps = psum.tile([128, 128], bf16, tag="pc", bufs=2)
            nc.tensor.transpose(tk_ps, kfin, ident_sb)
            nc.scalar.copy(out=kT_sb[:, st * 128:(st + 1) * 128], in_=tk_ps)

        # ================= out-projection units ====================
        # one unit = one [128,512] ec-chunk of y for one s-tile: 4 matmuls
        # accumulating over this core's 4 heads in a single psum bank, then
        # one copy to SBUF (bf16) and per-half (or per-unit for the final
        # qc) DMA out. Units are spliced between attention kt-iterations to
        # fill the PE idle left by the ACT-bound exp chain. Copies go to the
        # DVE except 1-in-4 (ACT is the att-block bottleneck engine).
        H_ORDER = (2, 3, 0, 1)  # h2 first: hp=1 normalizes first (hp order 1,0)

        def wout_units(qc):
            oT = ot_roll[qc]
            for sl in range(4):
                st = 4 * qc + sl
                out_sb = opool.tile([128, D], bf16, name="out_sb")
                for half in range(2):
                    for i in range(4):
                        ec = half * 4 + i
                        o_ps = psum.tile([128, 512], f32, tag="pd", bufs=2,
                                         name="o_ps")
                        for h in H_ORDER:
                            nc.tensor.matmul(
                                o_ps,
                                oT[:, h, sl * 128:(sl + 1) * 128],
                                wo_t[h][:, ec * 512:(ec + 1) * 512],
                                start=(h == H_ORDER[0]), stop=(h == H_ORDER[-1]))
                        if i == 3:
                            nc.scalar.copy(
                                out=out_sb[:, ec * 512:(ec + 1) * 512], in_=o_ps)
                        else:
                            nc.vector.tensor_copy(
                                out=out_sb[:, ec * 512:(ec + 1) * 512], in_=o_ps)
                        if qc == QC - 1:
                            nc.sync.dma_start(
                                out=out_d[st * 128:(st + 1) * 128,
                                          ec * 512:(ec + 1) * 512],
                                in_=out_sb[:, ec * 512:(ec + 1) * 512])
                        yield
                    if qc != QC - 1:
                        nc.sync.dma_start(
                            out=out_d[st * 128:(st + 1) * 128,
                                      half * 2048:(half + 1) * 2048],
                            in_=out_sb[:, half * 2048:(half + 1) * 2048])

        def drain(gen, n=10 ** 9):
            if gen is None:
                return True
            for _ in range(n):
                try:
                    next(gen)
                except StopIteration:
                    return True
            return False

        # ================= attention ====================
        # Splice drains sit BETWEEN the score/exp emission and the PV
        # matmuls: in the PE's static order the filler then executes exactly
        # where the PE would otherwise wait for the ACT exp chain.
        def att(qc, splice=None, pre=0, per_iter=0, hp_pre=0, den_pre=0):
            ot_roll[qc] = otpool.tile([128, HG, 512], bf16, name="ot")
            qt = qt_roll.pop(qc)
            drain(splice, pre)
            for hp in (1, 0):
                hh0 = 2 * hp
                if hp == 0:
                    drain(splice, hp_pre)
                pv0 = psum.tile([128, 512], f32, tag="pa", bufs=2, name="pv0")
                pv1 = psum.tile([128, 512], f32, tag="pa", bufs=2, name="pv1")
                # softmax denominator: accumulate exp tiles on DVE (bf16 =
                # 2x DVE rate), partition-reduce at the end via one
                # ones-matmul -> [128 identical rows, 512] broadcast.
                ea0 = accpool.tile([128, 512], bf16, name="ea0")
                ea1 = accpool.tile([128, 512], bf16, name="ea1")
                pvs, eas = [pv0, pv1], [ea0, ea1]
                n_kt = 4 * qc + 4
                for kt in range(n_kt):
                    j = kt - 4 * qc
                    off = 0 if j < 0 else 128 * j
                    exs = []
                    for hi in range(2):
                        h = hh0 + hi
                        # two score streams alternate pc/pb slots outside
                        # att(0) (pb holds the spliced qkv accumulators
                        # during att(0) itself).
                        sctag = "pc" if (hi == 0 or qc == 0) else "pb"
                        sc_ps = psum.tile([128, 512], f32, tag=sctag, bufs=2,
                                          name="sc")
                        nc.tensor.matmul(
                            sc_ps[:, off:512],
                            kT_sb[:, kt * 128:(kt + 1) * 128],
                            qt[:, h, off:512],
                            start=True, stop=True)
                        ex = epool.tile([128, 512], bf16, name=f"ex_{hi}")
                        nc.scalar.activation(out=ex[:, off:512],
                                             in_=sc_ps[:, off:512], func=AF.Exp)
                        if j >= 0:
                            nc.vector.tensor_mul(ex[:, off:off + 128],
                                                 ex[:, off:off + 128], mask_sb)
                        if kt == 0:
                            nc.vector.tensor_copy(out=eas[hi], in_=ex)
                        else:
                            nc.vector.tensor_add(eas[hi][:, off:512],
                                                 eas[hi][:, off:512],
                                                 ex[:, off:512])
                        exs.append(ex)
                    drain(splice, per_iter)
                    for hi in range(2):
                        nc.tensor.matmul(pvs[hi][:, off:512], v_sb[:, kt, :],
                                         exs[hi][:, off:512],
                                         start=(kt == 0), stop=(kt == n_kt - 1))
                drain(splice, den_pre)
                for hi in range(2):
                    h = hh0 + hi
                    den_ps = psum.tile([128, 512], f32, tag="pd", bufs=2,
                                       name="den")
                    nc.tensor.matmul(den_ps, onesm_sb, eas[hi],
                                     start=True, stop=True)
                    rden = scratch.tile([128, 512], f32, tag="rden")
                    nc.vector.reciprocal_approx_fast(out=rden, in_=den_ps)
                    nc.vector.tensor_mul(ot_roll[qc][:, h, :], pvs[hi], rden)

        # ================= fused schedule ====================
        # post_calc right after each tile's matmuls (frees PSUM accumulators
        # before any att block -- holding them across one deadlocks the slot
        # rotation); transposes lag one tile; wout units spliced into the
        # following att block to fill the exp-bound PE idle; qkv tile 5
        # spliced into att(0) (which has no wout work available yet).
        # mm3/mm4 are emitted BEFORE the prologue post_calcs: the scheduler
        # builds a static per-engine order from emission priority, and the PE
        # must have matmul work queued ahead of the transposes while the
        # three serial postproc chains drain on the DVE.
        def post_calc(st):
            post_cast(st)
            post_math(st)

        mm_group(); mm_tile(3, qtag="pc", kvtag="pc"); mm_tile(4)
        post_cast(0); post_cast(1); post_cast(2)
        post_math(0); post_cast(3)
        post_math(1); post_cast(4)
        post_math(2); post_math(3); post_math(4)
        post_transp(0); post_transp(1); post_transp(2); post_transp(3)
        mm5 = mm_tile_gen(5)
        att(0, splice=mm5, per_iter=4, hp_pre=0, den_pre=0)
        drain(mm5)
        post_transp(4)
        post_calc(5)
        mm_tile(6); post_calc(6); post_transp(5)
        mm_tile(7); post_calc(7); post_transp(6)
        mm_tile(8); post_calc(8); post_transp(7)
        w0 = wout_units(0)
        att(1, splice=w0, pre=3, per_iter=1, hp_pre=2, den_pre=1)
        drain(w0)
        post_transp(8)
        mm_tile(9); post_calc(9)
        mm_tile(10); post_calc(10); post_transp(9)
        mm_tile(11); post_calc(11); post_transp(10)
        mm_tile(12); post_calc(12); post_transp(11)
        w1 = wout_units(1)
        att(2, splice=w1, pre=3, per_iter=1, hp_pre=2, den_pre=1)
        drain(w1)
        post_transp(12)
        mm_tile(13); post_calc(13)
        mm_tile(14); post_calc(14); post_transp(13)
        mm_tile(15); post_calc(15); post_transp(14)
        post_transp(15)
        w2 = wout_units(2)
        att(3, splice=w2, pre=3, per_iter=1, hp_pre=2, den_pre=1)
        drain(w2)
        w3 = wout_units(3)
        drain(w3)

    nc.compile()
    return nc


def _host_prep(x, w_qkv, w_out, q_ln_w, k_ln_w):
    """Build per-core input maps (host-side shard + transform)."""
    import ml_dtypes
    bf16 = ml_dtypes.bfloat16

    x2 = np.asarray(x, np.float32).reshape(S, D)
    # x tiles [st, d_local, d_tile, s_local] so each s-tile DMA is contiguous
    xt = np.ascontiguousarray(
        x2.reshape(ST, 128, DT, 128).transpose(0, 3, 2, 1)).astype(bf16)

    # rope tables (duplicated cos / sign-baked sin, interleaved layout)
    freqs = 1.0 / (THETA ** (np.arange(0, HD, 2, dtype=np.float64) / HD))
    ang = np.arange(S, dtype=np.float64)[:, None] * freqs[None, :]
    cos = np.cos(ang).astype(np.float32)
    sin = np.sin(ang).astype(np.float32)
    ccd = np.repeat(cos, 2, axis=1).astype(np.float16)    # [S, 128]
    ssd = np.stack([-sin, sin], axis=-1).reshape(S, HD).astype(np.float16)

    kq = np.arange(128)
    dmask = (kq[:, None] <= kq[None, :]).astype(bf16)     # [k, q]
    ident = np.eye(128, dtype=bf16)

    wq = np.asarray(w_qkv, np.float32)
    wo = np.asarray(w_out, np.float32)
    qw = np.asarray(q_ln_w, np.float32)
    kw = np.asarray(k_ln_w, np.float32)

    in_maps = []
    for g in range(N_CORES):
        wq_g = wq[512 * g:512 * (g + 1), :].reshape(HG, HD, D) * qw[None, :, None]
        wk_g = wq[D + 128 * g:D + 128 * (g + 1), :] * kw[:, None]
        wv_g = wq[D + G * HD + 128 * g:D + G * HD + 128 * (g + 1), :]
        wqkv_g = np.concatenate([wq_g.reshape(512, D), wk_g, wv_g], axis=0)
        wqkvT_g = np.ascontiguousarray(wqkv_g.T).astype(bf16)     # [D, 768]
        woT_g = np.ascontiguousarray(wo[:, 512 * g:512 * (g + 1)].T).astype(bf16)
        in_maps.append({
            "xt": xt,
            "wqkvT": wqkvT_g,
            "woT": woT_g,
            "ccd": ccd,
            "ssd": ssd,
            "dmask": dmask,
            "ident": ident,
        })
    return in_maps


_CACHE = {}


def _get_compiled():
    if "nc" not in _CACHE:
        _ensure_ntff_hook()
        _CACHE["nc"] = _build_nc()
    return _CACHE["nc"]


def run_sharded(x, w_qkv, w_out, q_ln_w, k_ln_w, trace=False):
    from concourse.bass_utils import run_bass_kernel_spmd
    nc = _get_compiled()
    in_maps = _host_prep(x, w_qkv, w_out, q_ln_w, k_ln_w)
    res = run_bass_kernel_spmd(nc, in_maps, core_ids=list(range(N_CORES)),
                               trace=trace)
    acc = np.zeros((S, D), np.float32)
    for i in range(N_CORES):
        acc += np.asarray(res.results[i]["out"], np.float32)
    return acc.reshape(1, S, D), res


def kernel(x, w_qkv, w_out, q_ln_w, k_ln_w):
    out, _ = run_sharded(x, w_qkv, w_out, q_ln_w, k_ln_w, trace=False)
    return out


# revision 9
# speedup vs baseline: 1.0591x; 1.0271x over previous
"""Trainium2 Bass kernel for GQA multi-head attention block (nn_MHA_68831145886222).

Computation (reference):
  qkv = x @ w_qkv.T ; split q[32 heads],k[8],v[8] (HD=128)
  q,k = rmsnorm(head_dim) -> rope(interleaved, theta=1e6)
  out = causal GQA attention (4 q heads per kv head)
  y   = (attn out) @ w_out.T

Sharding: tensor-parallel by kv-head group. Core g of 8 owns q heads
4g..4g+3 and kv head g (columns of the qkv projection), plus the matching
512 input rows of w_out. Each core computes a partial y [2048,4096]; the
host sums the 8 partials.

Device-side layout choices per core:
  stage 1 (qkv proj):  stationary = x^T tiles [128 d, 128 s] (bf16),
                       moving = w_qkv^T slices -> qkv in natural [s, e] psum
  postproc: rmsnorm stats via DVE Square+reduce; rope via pair-swap copy +
            two muls + add (tables host-precomputed); per-head rstd apply
            (score scale 1/sqrt(HD) and ln weights folded in); PE-transpose
            q,k to [hd, s]; v stays natural [s, hd].
  attention: scoresT [k, q] = kT-tile.T @ qT (exact causal via restricted
             moving dim); exp on ACT; diagonal 128x128 blocks masked by a
             0/1 mask mul; PV and the softmax denominator (ones-matmul)
             accumulate over k-tiles in PSUM; normalize after PV.
  stage 3 (out proj): stationary = attnT [128 hd, 128 s], moving = w_out^T
             slices; accumulate over the core's 4 heads; fp32 partial out.

Scheduling: the PE is the bottleneck engine (~92% busy in the 396us
baseline); the remaining idle was concentrated at att(0) (ACT exp-bound,
nothing to splice -> HAM re-throttle) and at attention block/hp
boundaries. This version splices qkv tile 5's matmuls INTO att(0), moves
all splice drains between the score and PV matmuls (where the PE would
otherwise wait on ACT), pre-drains wout units at block/hp/den boundaries,
and re-tags PSUM so the spliced accumulators get dedicated banks
(pa/pb/pc/pd all bufs=2 = 8 banks) without slot-rotation deadlocks.
"""

import os
import sys
import types

import numpy as np

H = 32
G = 8
HD = 128
S = 2048
D = 4096
HG = H // G  # q heads per kv head = 4
EPS = 1e-5
THETA = 1e6
N_CORES = 8
ST = S // 128  # 16 s-tiles
DT = D // 128  # 32 d-tiles
QC = 4  # q chunks of 512
EC = 8  # e chunks of 512 in final matmul


def _ensure_ntff_hook():
    """Register the axon NTFF profile hook if the image's antenv lacks it,
    so run_bass_kernel_spmd(trace=True) can return exec_time_ns."""
    try:
        from antenv.axon_hooks import get_axon_ntff_profile_hook  # noqa: F401
        return
    except ImportError:
        pass
    try:
        import antenv
        mod = types.ModuleType("antenv.axon_hooks")
        _h = [None]
        mod.set_axon_ntff_profile_hook = lambda h: _h.__setitem__(0, h)
        mod.get_axon_ntff_profile_hook = lambda: _h[0]
        sys.modules["antenv.axon_hooks"] = mod
        antenv.axon_hooks = mod
        from trn_agent_boot.trn_boot import _ntff_profile_via_ctypes
        so = "/opt/axon/libaxon_pjrt.so"
        if os.path.exists(so):
            mod.set_axon_ntff_profile_hook(_ntff_profile_via_ctypes(so))
    except Exception:
        pass


def _build_nc():
    import concourse.bass as bass  # noqa: F401
    import concourse.tile as tile
    from concourse import bacc, mybir

    bf16 = mybir.dt.bfloat16
    f16 = mybir.dt.float16
    f32 = mybir.dt.float32
    i32 = mybir.dt.int32
    AF = mybir.ActivationFunctionType

    nc = bacc.Bacc("TRN2", target_bir_lowering=False, debug=False,
                   num_devices=N_CORES)

    # ---- DRAM I/O ----
    xt_d = nc.dram_tensor("xt", [ST, 128, DT, 128], bf16, kind="ExternalInput").ap()
    wqkv_d = nc.dram_tensor("wqkvT", [D, 768], bf16, kind="ExternalInput").ap()
    wo_d = nc.dram_tensor("woT", [512, D], bf16, kind="ExternalInput").ap()
    ccd_d = nc.dram_tensor("ccd", [S, 128], f16, kind="ExternalInput").ap()
    ssd_d = nc.dram_tensor("ssd", [S, 128], f16, kind="ExternalInput").ap()
    mask_d = nc.dram_tensor("dmask", [128, 128], bf16, kind="ExternalInput").ap()
    ident_d = nc.dram_tensor("ident", [128, 128], bf16, kind="ExternalInput").ap()
    out_d = nc.dram_tensor("out", [S, D], bf16, kind="ExternalOutput").ap()

    from contextlib import ExitStack
    with tile.TileContext(nc) as tc, ExitStack() as ctx:
        const = ctx.enter_context(tc.tile_pool(name="const", bufs=1))
        persist = ctx.enter_context(tc.tile_pool(name="persist", bufs=1))
        xpool = ctx.enter_context(tc.tile_pool(name="xpool", bufs=2))
        scratch = ctx.enter_context(tc.tile_pool(name="scratch", bufs=2))
        small = ctx.enter_context(tc.tile_pool(name="small", bufs=2))
        epool = ctx.enter_context(tc.tile_pool(name="epool", bufs=5))
        accpool = ctx.enter_context(tc.tile_pool(name="accpool", bufs=2))
        qtpool = ctx.enter_context(tc.tile_pool(name="qtpool", bufs=2))
        otpool = ctx.enter_context(tc.tile_pool(name="otpool", bufs=2))
        opool = ctx.enter_context(tc.tile_pool(name="opool", bufs=2))
        psum = ctx.enter_context(tc.tile_pool(name="psum", bufs=2, space="PSUM"))

        # ---- critical path to first matmul: first x chunk + first wq chunk --
        # wq is split into 16 chunk tiles of 2 dt each so the first matmul
        # only depends on chunk 0 (per-tile dependency granularity).
        wq_r = wqkv_d.rearrange("(t p) e -> p t e", p=128)
        wq_t = [persist.tile([128, 2, 768], bf16, name=f"wq{c}")
                for c in range(DT // 2)]
        # x tiles for the dt-major prologue group (st 0-2) are split into
        # per-chunk tiles and interleaved with the wq chunks in DMA issue
        # order, matching the dt-major consumption pattern.
        XS0_CHUNKS = [(0, 4), (4, 8), (8, 16), (16, 24), (24, 32)]
        xsg_t = {}
        for g in range(3):
            xsg_t[g] = [xpool.tile([128, c1 - c0, 128], bf16,
                                   name=f"xs{g}_{i}", bufs=1)
                        for i, (c0, c1) in enumerate(XS0_CHUNKS)]
        nc.sync.dma_start(out=xsg_t[0][0], in_=xt_d[0, :, 0:4, :])
        nc.sync.dma_start(out=wq_t[0], in_=wq_r[:, 0:2, :])
        nc.sync.dma_start(out=xsg_t[1][0], in_=xt_d[1, :, 0:4, :])
        nc.sync.dma_start(out=xsg_t[2][0], in_=xt_d[2, :, 0:4, :])
        nc.sync.dma_start(out=wq_t[1], in_=wq_r[:, 2:4, :])
        for i, (c0, c1) in enumerate(XS0_CHUNKS):
            if i > 0:
                for g in range(3):
                    nc.sync.dma_start(out=xsg_t[g][i], in_=xt_d[g, :, c0:c1, :])
            w0, w1 = (2, 4) if i == 1 else ((4 * i - 4, 4 * i) if i >= 2 else (0, 0))
            for c in range(w0, w1):
                nc.sync.dma_start(out=wq_t[c], in_=wq_r[:, 2 * c:2 * c + 2, :])
        # pre-issue xs3/xs4 so their data lands right as the prologue group
        # finishes (before the rope tables and wo, which are needed later).
        xs_pre = {}
        for st0 in (3, 4):
            xs_p = xpool.tile([128, DT, 128], bf16, name="xs")
            nc.sync.dma_start(out=xs_p, in_=xt_d[st0])
            xs_pre[st0] = xs_p

        # ---- constants / persistent tensors ----
        ccd_sb = const.tile([128, ST, 128], f16)
        nc.sync.dma_start(out=ccd_sb, in_=ccd_d.rearrange("(t p) h -> p t h", p=128))
        ssd_sb = const.tile([128, ST, 128], f16)
        nc.sync.dma_start(out=ssd_sb, in_=ssd_d.rearrange("(t p) h -> p t h", p=128))
        mask_sb = const.tile([128, 128], bf16)
        nc.sync.dma_start(out=mask_sb, in_=mask_d)
        ident_sb = const.tile([128, 128], bf16)
        nc.sync.dma_start(out=ident_sb, in_=ident_d)
        onesm_sb = const.tile([128, 128], bf16)
        nc.vector.memset(onesm_sb, 1.0)
        onesw_sb = const.tile([128, 512], bf16)
        nc.vector.memset(onesw_sb, 1.0)

        # xs5 pre-issue after the tables (tile 5 is consumed early -- spliced
        # into att(0) -- but the rope tables are needed earlier still).
        xs5_p = xpool.tile([128, DT, 128], bf16, name="xs")
        nc.sync.dma_start(out=xs5_p, in_=xt_d[5])
        xs_pre[5] = xs5_p

        # warmup matmuls during the initial DMA wait: ~6us of PE activity
        # trips the HAM clock gate to 8/8 before the real matmuls start
        # (otherwise the first ~3.4us of stage 1 runs at 1.2 GHz). N=512
        # moving keeps the PE busy-density high per instruction.
        for w in range(16):
            warm_ps = psum.tile([128, 512], f32, tag="pb", bufs=2, name="warm")
            nc.tensor.matmul(warm_ps, onesm_sb, onesw_sb, start=True, stop=True)

        # stage-3 weights: needed from the first wout units (~150us in); per-h
        # chunk tiles so the first unit (H_ORDER starts at h=2) doesn't wait
        # for the full 4.2 MB.
        wo_r = wo_d.rearrange("(h p) e -> h p e", p=128)
        wo_t = [persist.tile([128, D], bf16, name=f"wo{h}") for h in range(HG)]
        for h in (2, 3, 0, 1):
            nc.sync.dma_start(out=wo_t[h], in_=wo_r[h])

        kT_sb = persist.tile([128, S], bf16)       # [hd, s]
        v_sb = persist.tile([128, ST, 128], bf16)  # [s_local, s_tile, hd]
        # rolling per-qc buffers (written by 4 s-tiles / one att block,
        # consumed one phase later)
        qt_roll = {}   # qc -> [128, HG, 512] bf16
        ot_roll = {}   # qc -> [128, HG, 512] bf16

        # ================= per-s-tile pieces =================
        state = {}  # st -> (q_ps, kv_ps)

        def xs_ap_for(st):
            if st < 3:
                chunks = []
                for i, (c0, c1) in enumerate(XS0_CHUNKS):
                    for d in range(c1 - c0):
                        chunks.append((xsg_t[st][i], d))
                return lambda dt_i: chunks[dt_i][0][:, chunks[dt_i][1], :]
            if st in xs_pre:
                xs = xs_pre[st]
            else:
                xs = xpool.tile([128, DT, 128], bf16, name="xs")
                nc.sync.dma_start(out=xs, in_=xt_d[st])
            return lambda dt_i: xs[:, dt_i, :]

        def mm_group():
            """dt-major qkv matmuls for s-tiles 0-2 together: cuts the
            wq-chunk consumption rate to a third so the PE never outruns the
            wq DMA, and banks ~31us of PE work before the first postproc
            chains need to finish."""
            aps = [xs_ap_for(st) for st in range(3)]
            qtags = ["pa", "pa", "pb"]
            qps = [psum.tile([128, 512], f32, tag=qtags[g], bufs=2,
                             name=f"q_ps{g}")
                   for g in range(3)]
            kvtags = ["pd", "pd", "pb"]
            kvps = [psum.tile([128, 512], f32, tag=kvtags[g], bufs=2,
                              name=f"kv_ps{g}")
                    for g in range(3)]
            for dt_i in range(DT):
                wq_c = wq_t[dt_i // 2][:, dt_i % 2, :]
                for g in range(3):
                    nc.tensor.matmul(qps[g], aps[g](dt_i), wq_c[:, 0:512],
                                     start=(dt_i == 0), stop=(dt_i == DT - 1))
                    nc.tensor.matmul(kvps[g][:, 0:256], aps[g](dt_i),
                                     wq_c[:, 512:768],
                                     start=(dt_i == 0), stop=(dt_i == DT - 1))
            for g in range(3):
                state[g] = (qps[g], kvps[g])

        def mm_tile(st, qtag="pa", kvtag="pd"):
            ap = xs_ap_for(st)
            q_ps = psum.tile([128, 512], f32, tag=qtag, bufs=2, name="q_ps")
            kv_ps = psum.tile([128, 512], f32, tag=kvtag, bufs=2, name="kv_ps")
            for dt_i in range(DT):
                wq_c = wq_t[dt_i // 2][:, dt_i % 2, :]
                nc.tensor.matmul(q_ps, ap(dt_i), wq_c[:, 0:512],
                                 start=(dt_i == 0), stop=(dt_i == DT - 1))
                nc.tensor.matmul(kv_ps[:, 0:256], ap(dt_i), wq_c[:, 512:768],
                                 start=(dt_i == 0), stop=(dt_i == DT - 1))
            state[st] = (q_ps, kv_ps)

        def mm_tile_gen(st):
            """Generator form of mm_tile for splicing into an att block: the
            accumulators live on the dedicated pb tag so the slot rotation of
            the att block's own psum tiles (pa/pc/pd) can't cycle on them."""
            ap = xs_ap_for(st)
            q_ps = psum.tile([128, 512], f32, tag="pb", bufs=2, name="q_ps")
            kv_ps = psum.tile([128, 512], f32, tag="pb", bufs=2, name="kv_ps")
            state[st] = (q_ps, kv_ps)
            for dt_i in range(DT):
                wq_c = wq_t[dt_i // 2][:, dt_i % 2, :]
                nc.tensor.matmul(q_ps, ap(dt_i), wq_c[:, 0:512],
                                 start=(dt_i == 0), stop=(dt_i == DT - 1))
                nc.tensor.matmul(kv_ps[:, 0:256], ap(dt_i), wq_c[:, 512:768],
                                 start=(dt_i == 0), stop=(dt_i == DT - 1))
                yield

        fins = {}  # st -> fin [128, 5, 128] (q heads 0-3, k at 4)
        casts = {}  # st -> qkb [128, 5, 128] f16

        def post_cast(st):
            """Cast q/k out of PSUM to f16 SBUF (frees the psum slots
            within ~1us so later tiles' matmuls can start). q heads and k
            land in one [128, 5, 128] tile so the whole rope/rmsnorm chain
            runs as single wide DVE ops over all 5 heads."""
            q_ps, kv_ps = state.pop(st)
            qkb = scratch.tile([128, 5, 128], f16, bufs=3, name="qkb")
            nc.vector.tensor_copy(out=qkb[:, 4, :], in_=kv_ps[:, 0:128])
            nc.vector.tensor_copy(out=v_sb[:, st, :], in_=kv_ps[:, 128:256])
            nc.vector.tensor_copy(
                out=qkb[:, 0:4, :],
                in_=q_ps.rearrange("p (h e) -> p h e", h=HG))
            casts[st] = qkb

        def post_math(st):
            """Rope + rmsnorm stats + final bf16 q/k tiles (all f16 DVE).
            One chain over [128, 5, 128] (4 q heads + k); the pair-swap for
            rope is a reversed-stride AP read folded into the sin mul."""
            qkb = casts.pop(st)
            cc_b = ccd_sb[:, st, :].unsqueeze(1).broadcast_to((128, 5, 128))
            ss_b = ssd_sb[:, st, :].unsqueeze(1).broadcast_to((128, 5, 128))
            qcc = scratch.tile([128, 5, 128], f16, name="qcc")
            nc.vector.tensor_mul(qcc, qkb, cc_b)
            qss = scratch.tile([128, 5, 128], f16, name="qss")
            nc.vector.tensor_mul(
                qss.rearrange("p h (r two) -> p h r two", two=2),
                qkb.rearrange("p h (r two) -> p h r two", two=2)[:, :, :, ::-1],
                ss_b.rearrange("p h (r two) -> p h r two", two=2))
            rope = scratch.tile([128, 5, 128], f16, name="rope")
            nc.vector.tensor_add(rope, qcc, qss)

            # sum of squares per head from the rope'd values (rope is a
            # per-pair rotation so the head norms are unchanged; keeps
            # Square off ACT so its table cache only holds {Copy, Exp}).
            # The squares overwrite qcc (dead after the rope add).
            nc.vector.tensor_mul(qcc, rope, rope)
            ssq = small.tile([128, 5], f32)
            nc.vector.tensor_reduce(
                out=ssq.rearrange("p (h one) -> p h one", one=1),
                in_=qcc, axis=mybir.AxisListType.X, op=mybir.AluOpType.add)
            # rstd = 1/sqrt(ssq*scale + eps) via DVE fast-inverse-sqrt (magic
            # seed + one Newton step, rel err ~2e-3). Keeps Sqrt off ACT so
            # its table cache only ever holds {Copy, Exp} -- no reloads.
            # q cols get the 1/sqrt(HD) score scale folded in.
            x5 = small.tile([128, 5], f32)
            nc.vector.tensor_scalar_add(x5[:, 0:4], ssq[:, 0:4],
                                        float(HD * EPS))
            nc.vector.tensor_scalar(out=x5[:, 4:5], in0=ssq[:, 4:5],
                                    scalar1=1.0 / HD, scalar2=float(EPS),
                                    op0=mybir.AluOpType.mult,
                                    op1=mybir.AluOpType.add)
            xi = x5.bitcast(i32)
            t5 = small.tile([128, 5], i32)
            nc.vector.tensor_scalar(out=t5, in0=xi, scalar1=1, scalar2=None,
                                    op0=mybir.AluOpType.arith_shift_right)
            # y0i = MAGIC - t  ==  (t ^ -1) + (MAGIC + 1)
            nt5 = small.tile([128, 5], i32)
            nc.vector.tensor_scalar(out=nt5, in0=t5, scalar1=-1, scalar2=None,
                                    op0=mybir.AluOpType.bitwise_xor)
            y0i = small.tile([128, 5], i32)
            nc.vector.tensor_scalar_add(y0i, nt5, 0x5f375a86 + 1)
            y0 = y0i.bitcast(f32)
            a5 = small.tile([128, 5], f32)
            nc.vector.tensor_mul(a5, x5, y0)
            b5 = small.tile([128, 5], f32)
            nc.vector.tensor_mul(b5, a5, y0)            # x*y0^2
            c5 = small.tile([128, 5], f32)
            nc.vector.tensor_scalar(out=c5, in0=b5, scalar1=-0.5, scalar2=1.5,
                                    op0=mybir.AluOpType.mult,
                                    op1=mybir.AluOpType.add)
            rstd = small.tile([128, 5], f32)
            nc.vector.tensor_mul(rstd, y0, c5)

            fin = scratch.tile([128, 5, 128], bf16, bufs=4, name="fin")
            nc.vector.tensor_mul(
                fin, rope, rstd.unsqueeze(2).broadcast_to((128, 5, 128)))
            fins[st] = fin

        def post_transp(st):
            """PE transposes of q/k into [hd, s]; lagged one s-tile so the
            post_calc chain hides under the next tile's matmuls."""
            fin = fins.pop(st)
            qc, sl = st // 4, st % 4
            if sl == 0:
                qt_roll[qc] = qtpool.tile([128, HG, 512], bf16, name="qt")
            for hh in range(HG):
                tq_ps = psum.tile([128, 128], bf16, tag="pc", bufs=2)
                nc.tensor.transpose(tq_ps, fin[:, hh, :], ident_sb)
                nc.scalar.copy(out=qt_roll[qc][:, hh, sl * 128:(sl + 1) * 128],
                               in_=tq_ps)
            tk_ps = psum.tile([128, 128], bf16, tag="pc", bufs=2)
            nc.tensor.transpose(tk_ps, fin[:, 4, :], ident_sb)
            nc.scalar.copy(out=kT_sb[:, st * 128:(st + 1) * 128], in_=tk_ps)

        # ================= out-projection units ====================
        # one unit = one [128,512] ec-chunk of y for one s-tile: 4 matmuls
        # accumulating over this core's 4 heads in a single psum bank, then
        # one copy to SBUF (bf16) and per-half (or per-unit for the final
        # qc) DMA out. Units are spliced between attention kt-iterations to
        # fill the PE idle left by the ACT-bound exp chain. The PSUM->SBUF
        # copy engine is phase-dependent (ueng): DVE for in-att per-iter
        # units (ACT is exp-bound there), ACT for boundary units (DVE is
        # grinding post_math chains there), alternating when both are free.
        H_ORDER = (2, 3, 0, 1)  # h2 first: hp=1 normalizes first (hp order 1,0)
        ueng = ['v']
        _alt = [0]

        def wout_units(qc):
            oT = ot_roll[qc]
            for sl in range(4):
                st = 4 * qc + sl
                out_sb = opool.tile([128, D], bf16, name="out_sb")
                for half in range(2):
                    for i in range(4):
                        ec = half * 4 + i
                        o_ps = psum.tile([128, 512], f32, tag="pd", bufs=2,
                                         name="o_ps")
                        for h in H_ORDER:
                            nc.tensor.matmul(
                                o_ps,
                                oT[:, h, sl * 128:(sl + 1) * 128],
                                wo_t[h][:, ec * 512:(ec + 1) * 512],
                                start=(h == H_ORDER[0]), stop=(h == H_ORDER[-1]))
                        e = ueng[0]
                        if e == 'a':
                            _alt[0] += 1
                            e = 's' if _alt[0] % 2 == 0 else 'v'
                        if e == 's':
                            nc.scalar.copy(
                                out=out_sb[:, ec * 512:(ec + 1) * 512], in_=o_ps)
                        else:
                            nc.vector.tensor_copy(
                                out=out_sb[:, ec * 512:(ec + 1) * 512], in_=o_ps)
                        if qc == QC - 1:
                            nc.sync.dma_start(
                                out=out_d[st * 128:(st + 1) * 128,
                                          ec * 512:(ec + 1) * 512],
                                in_=out_sb[:, ec * 512:(ec + 1) * 512])
                        yield
                    if qc != QC - 1:
                        nc.sync.dma_start(
                            out=out_d[st * 128:(st + 1) * 128,
                                      half * 2048:(half + 1) * 2048],
                            in_=out_sb[:, half * 2048:(half + 1) * 2048])

        def drain(gen, n=10 ** 9):
            if gen is None:
                return True
            for _ in range(n):
                try:
                    next(gen)
                except StopIteration:
                    return True
            return False

        # ================= attention ====================
        # Splice drains sit BETWEEN the score/exp emission and the PV
        # matmuls: in the PE's static order the filler then executes exactly
        # where the PE would otherwise wait for the ACT exp chain. Both hi
        # streams share one [128, 2, 512] ex/ea tile so the mask mul and
        # denominator accumulation run as single wide DVE ops per kt.
        mask_b = mask_sb.unsqueeze(1).broadcast_to((128, 2, 128))

        def att(qc, splice=None, pre=0, per_iter=0, hp_pre=0, den_pre=0,
                skip_every=0):
            ot_roll[qc] = otpool.tile([128, HG, 512], bf16, name="ot")
            qt = qt_roll.pop(qc)
            ueng[0] = 's'
            drain(splice, pre)
            ueng[0] = 'v'
            it = 0
            for hp in (1, 0):
                hh0 = 2 * hp
                if hp == 0:
                    ueng[0] = 's'
                    drain(splice, hp_pre)
                    ueng[0] = 'v'
                pv0 = psum.tile([128, 512], f32, tag="pa", bufs=2, name="pv0")
                pv1 = psum.tile([128, 512], f32, tag="pa", bufs=2, name="pv1")
                pvs = [pv0, pv1]
                # softmax denominator: accumulate exp tiles on DVE (bf16 =
                # 2x DVE rate), partition-reduce at the end via one
                # ones-matmul -> [128 identical rows, 512] broadcast.
                ea = accpool.tile([128, 2, 512], bf16, name="ea")
                n_kt = 4 * qc + 4
                for kt in range(n_kt):
                    j = kt - 4 * qc
                    off = 0 if j < 0 else 128 * j
                    ex = epool.tile([128, 2, 512], bf16, name="ex")
                    for hi in range(2):
                        h = hh0 + hi
                        sc_ps = psum.tile([128, 512], f32, tag="pc", bufs=2,
                                          name="sc")
                        nc.tensor.matmul(
                            sc_ps[:, off:512],
                            kT_sb[:, kt * 128:(kt + 1) * 128],
                            qt[:, h, off:512],
                            start=True, stop=True)
                        nc.scalar.activation(out=ex[:, hi, off:512],
                                             in_=sc_ps[:, off:512], func=AF.Exp)
                    if j >= 0:
                        nc.vector.tensor_mul(ex[:, :, off:off + 128],
                                             ex[:, :, off:off + 128], mask_b)
                    if kt == 0:
                        nc.vector.tensor_copy(out=ea, in_=ex)
                    else:
                        nc.vector.tensor_add(ea[:, :, off:512],
                                             ea[:, :, off:512],
                                             ex[:, :, off:512])
                    if skip_every == 0 or (it % skip_every) != skip_every - 1:
                        drain(splice, per_iter)
                    it += 1
                    for hi in range(2):
                        nc.tensor.matmul(pvs[hi][:, off:512], v_sb[:, kt, :],
                                         ex[:, hi, off:512],
                                         start=(kt == 0), stop=(kt == n_kt - 1))
                ueng[0] = 's'
                drain(splice, den_pre)
                ueng[0] = 'v'
                for hi in range(2):
                    h = hh0 + hi
                    den_ps = psum.tile([128, 512], f32, tag="pd", bufs=2,
                                       name="den")
                    nc.tensor.matmul(den_ps, onesm_sb, ea[:, hi, :],
                                     start=True, stop=True)
                    rden = scratch.tile([128, 512], f32, tag="rden")
                    nc.vector.reciprocal_approx_fast(out=rden, in_=den_ps)
                    nc.vector.tensor_mul(ot_roll[qc][:, h, :], pvs[hi], rden)

        # ================= fused schedule ====================
        # post_calc right after each tile's matmuls (frees PSUM accumulators
        # before any att block -- holding them across one deadlocks the slot
        # rotation); transposes lag one tile; wout units spliced into the
        # following att block to fill the exp-bound PE idle; qkv tile 5
        # spliced into att(0) (which has no wout work available yet).
        # mm3/mm4 are emitted BEFORE the prologue post_calcs: the scheduler
        # builds a static per-engine order from emission priority, and the PE
        # must have matmul work queued ahead of the transposes while the
        # three serial postproc chains drain on the DVE.
        def post_calc(st):
            post_cast(st)
            post_math(st)

        mm_group(); mm_tile(3, qtag="pc", kvtag="pc"); mm_tile(4)
        post_cast(0); post_cast(1); post_cast(2)
        post_math(0); post_cast(3)
        post_math(1); post_cast(4)
        post_math(2); post_math(3); post_math(4)
        post_transp(0); post_transp(1); post_transp(2); post_transp(3)
        mm5 = mm_tile_gen(5)
        att(0, splice=mm5, per_iter=4)
        drain(mm5)
        post_transp(4)
        post_calc(5)
        # the mm tile right after an att block goes on pb (free outside
        # att(0)): its accumulators then wait only on the previous spliced
        # tile's cast, not on the att block's den/ot DVE chain via pa/pd
        # slot rotation.
        mm_tile(6, qtag="pb", kvtag="pb"); post_calc(6); post_transp(5)
        mm_tile(7); post_calc(7); post_transp(6)
        mm_tile(8); post_calc(8); post_transp(7)
        w0 = wout_units(0)
        att(1, splice=w0, pre=3, per_iter=1, hp_pre=2, den_pre=1)
        ueng[0] = 'a'
        drain(w0)
        post_transp(8)
        mm_tile(9, qtag="pb", kvtag="pb"); post_calc(9)
        mm_tile(10); post_calc(10); post_transp(9)
        mm_tile(11); post_calc(11); post_transp(10)
        mm_tile(12); post_calc(12); post_transp(11)
        w1 = wout_units(1)
        # att(2) takes filler every other iteration only, leaving 13 w1
        # units to cover the mm13-15 / transpose-15 boundary before att(3)
        # (where the tile-15 cast+math+transpose chain would starve the PE).
        att(2, splice=w1, pre=3, per_iter=1, skip_every=2, hp_pre=2, den_pre=1)
        ueng[0] = 's'
        drain(w1, 2)
        post_transp(12)
        mm_tile(13, qtag="pb", kvtag="pb"); post_calc(13)
        mm_tile(14); post_calc(14); post_transp(13)
        mm_tile(15); post_calc(15)
        post_transp(14)
        ueng[0] = 's'
        drain(w1, 4)
        post_transp(15)
        ueng[0] = 's'
        drain(w1)
        w2 = wout_units(2)
        att(3, splice=w2, per_iter=1, skip_every=8, hp_pre=2, den_pre=1)
        ueng[0] = 'a'
        drain(w2)
        w3 = wout_units(3)
        drain(w3)

    nc.compile()
    return nc


def _host_prep(x, w_qkv, w_out, q_ln_w, k_ln_w):
    """Build per-core input maps (host-side shard + transform)."""
    import ml_dtypes
    bf16 = ml_dtypes.bfloat16

    x2 = np.asarray(x, np.float32).reshape(S, D)
    # x tiles [st, d_local, d_tile, s_local] so each s-tile DMA is contiguous
    xt = np.ascontiguousarray(
        x2.reshape(ST, 128, DT, 128).transpose(0, 3, 2, 1)).astype(bf16)

    # rope tables (duplicated cos / sign-baked sin, interleaved layout)
    freqs = 1.0 / (THETA ** (np.arange(0, HD, 2, dtype=np.float64) / HD))
    ang = np.arange(S, dtype=np.float64)[:, None] * freqs[None, :]
    cos = np.cos(ang).astype(np.float32)
    sin = np.sin(ang).astype(np.float32)
    ccd = np.repeat(cos, 2, axis=1).astype(np.float16)    # [S, 128]
    ssd = np.stack([-sin, sin], axis=-1).reshape(S, HD).astype(np.float16)

    kq = np.arange(128)
    dmask = (kq[:, None] <= kq[None, :]).astype(bf16)     # [k, q]
    ident = np.eye(128, dtype=bf16)

    wq = np.asarray(w_qkv, np.float32)
    wo = np.asarray(w_out, np.float32)
    qw = np.asarray(q_ln_w, np.float32)
    kw = np.asarray(k_ln_w, np.float32)

    in_maps = []
    for g in range(N_CORES):
        wq_g = wq[512 * g:512 * (g + 1), :].reshape(HG, HD, D) * qw[None, :, None]
        wk_g = wq[D + 128 * g:D + 128 * (g + 1), :] * kw[:, None]
        wv_g = wq[D + G * HD + 128 * g:D + G * HD + 128 * (g + 1), :]
        wqkv_g = np.concatenate([wq_g.reshape(512, D), wk_g, wv_g], axis=0)
        wqkvT_g = np.ascontiguousarray(wqkv_g.T).astype(bf16)     # [D, 768]
        woT_g = np.ascontiguousarray(wo[:, 512 * g:512 * (g + 1)].T).astype(bf16)
        in_maps.append({
            "xt": xt,
            "wqkvT": wqkvT_g,
            "woT": woT_g,
            "ccd": ccd,
            "ssd": ssd,
            "dmask": dmask,
            "ident": ident,
        })
    return in_maps


_CACHE = {}


def _get_compiled():
    if "nc" not in _CACHE:
        _ensure_ntff_hook()
        _CACHE["nc"] = _build_nc()
    return _CACHE["nc"]


def run_sharded(x, w_qkv, w_out, q_ln_w, k_ln_w, trace=False):
    from concourse.bass_utils import run_bass_kernel_spmd
    nc = _get_compiled()
    in_maps = _host_prep(x, w_qkv, w_out, q_ln_w, k_ln_w)
    res = run_bass_kernel_spmd(nc, in_maps, core_ids=list(range(N_CORES)),
                               trace=trace)
    acc = np.zeros((S, D), np.float32)
    for i in range(N_CORES):
        acc += np.asarray(res.results[i]["out"], np.float32)
    return acc.reshape(1, S, D), res


def kernel(x, w_qkv, w_out, q_ln_w, k_ln_w):
    out, _ = run_sharded(x, w_qkv, w_out, q_ln_w, k_ln_w, trace=False)
    return out
